# revision 12
# baseline (speedup 1.0000x reference)
"""Multi-head attention (RoPE + causal SDPA) on 8 Trainium2 NeuronCores.

Sharding: tensor-parallel over heads for QKV+attention (2 heads/core),
then an on-device AllToAll (one per batch half, so comm overlaps other
compute) reshards from head-split to row-split, and each core computes
its row slice of the output projection with the full wo. Host side only
slices/transposes/concatenates.

Everything is computed "transposed" (feature dims on SBUF partitions,
sequence on the free axis), so no transposes are needed on the hot path:
  Qt/Kt  (128=2*64 head dims, S)  = W_slice @ x.T   (lhsT=W_sliceT chunks)
  scores (128 kpos, <=512 q)      lhsT=Kt block, rhs=Qt slice; the two
                                  heads' score matmuls use disjoint PE
                                  row-groups (contraction 64 each) so
                                  they run concurrently, into one
                                  [128, 2*512] PSUM pair tile
  P.T    = exp(scores/8)          one ScalarE ACTIVATE per pair (3D AP)
  O.T    (64+1, <=512 q)          accumulated over k blocks in PSUM;
                                  row 64 = softmax sum (ones col in V)
  out.T  (1024 e, rows)           = woT chunks.T @ O_full.T

Causal structure: k-blocks at/above the q-tile diagonal are trimmed to
the needed column suffix (score MM, exp, PV all shrink); the partial
triangle gets a single [128,128] mask multiply per head.

Emission interleaves batch-1 projections with batch-0 attention (and
out-proj with batch-1 attention) so the PE always has dense independent
work while ScalarE chews exp -- keeps the HAM clock gate at full rate.
"""
import sys, os
if '/opt/trn_rl_repo' not in sys.path:
    sys.path.insert(0, '/opt/trn_rl_repo')
os.environ.setdefault('MYCRO_LOCAL_CACHE', '1')

from contextlib import ExitStack

import numpy as np
import ml_dtypes

import concourse.bass as bass
import concourse.tile as tile
from concourse import bacc, mybir
from concourse.bass_utils import run_bass_kernel_spmd
from concourse.masks import make_identity

BF16 = ml_dtypes.bfloat16
NC = 8           # cores
B = 2            # batch
D = 1024         # model dim
H = 16           # heads
HD = 64          # head dim
HPC = H // NC    # heads per core = 2
DPC = HPC * HD   # head dims per core = 128
ROPE_BASE = 10000.0
QT = 512         # projection tile / attention q tile
KB = 128         # k block (partition axis of score matmuls)

F32 = mybir.dt.float32
BF = mybir.dt.bfloat16
MULT = mybir.AluOpType.mult
ADD = mybir.AluOpType.add
EXP = mybir.ActivationFunctionType.Exp


def build_nc(S):
    """Build+compile the SPMD Bass module for sequence length S."""
    RH = S // NC           # rows per core per batch
    NST = S // QT          # 512-wide seq tiles per batch
    NVT = S // KB          # V tiles (of 128 kpos) per batch
    DCH = D // 128         # contraction chunks (= 8)
    NHALF = 2              # collectives per batch (row halves)
    RHH = RH // NHALF      # rows per core per half

    nc = bacc.Bacc(num_devices=NC)

    xT = nc.declare_dram_parameter("xT", [D, B * S], BF, isOutput=False)
    wqT = nc.declare_dram_parameter("wqT", [D, DPC], BF, isOutput=False)
    wkT = nc.declare_dram_parameter("wkT", [D, DPC], BF, isOutput=False)
    wvT = nc.declare_dram_parameter("wvT", [D, DPC], BF, isOutput=False)
    woT = nc.declare_dram_parameter("woT", [D, D], BF, isOutput=False)
    cosT = nc.declare_dram_parameter("cosT", [128, S], BF, isOutput=False)
    sinT = nc.declare_dram_parameter("sinT", [128, S], BF, isOutput=False)
    mask128 = nc.declare_dram_parameter("mask128", [128, KB], BF,
                                        isOutput=False)
    out = nc.declare_dram_parameter("out", [D, B * RH], F32, isOutput=True)

    a2a_in = {(b, h): nc.dram_tensor(f"a2a_in{b}_{h}", [NC, 128, RHH], BF)
              for b in range(B) for h in range(NHALF)}
    a2a_out = {(b, h): nc.dram_tensor(f"a2a_out{b}_{h}", [NC, 128, RHH], BF)
               for b in range(B) for h in range(NHALF)}

    ctx = ExitStack()
    with ctx:
        tc = ctx.enter_context(tile.TileContext(nc))

        consts = ctx.enter_context(tc.tile_pool(name="consts", bufs=1))
        xpool = ctx.enter_context(tc.tile_pool(name="x", bufs=2 * DCH))
        pQt = ctx.enter_context(tc.tile_pool(name="qt", bufs=2))
        pKt = ctx.enter_context(tc.tile_pool(name="kt", bufs=2))
        pV = ctx.enter_context(tc.tile_pool(name="v", bufs=2))
        pO = ctx.enter_context(tc.tile_pool(name="oall", bufs=2))
        ptmp = ctx.enter_context(tc.tile_pool(name="tmp", bufs=3))
        praw = ctx.enter_context(tc.tile_pool(name="raw", bufs=2))
        ppt = ctx.enter_context(tc.tile_pool(name="pt", bufs=4))
        pnorm = ctx.enter_context(tc.tile_pool(name="norm", bufs=4))
        pog = ctx.enter_context(tc.tile_pool(name="og", bufs=3 * NC))
        posb = ctx.enter_context(tc.tile_pool(name="osb", bufs=2))

        # PSUM: 8 banks total.
        # ppA (2 x 1 bank): projections + PE-transposes + out-proj
        # pps (2 x 2 banks): score pair tiles (128, 2*512) f32
        # ppo (2 x 1 bank): O.T accumulators (128, 512)
        ppA = ctx.enter_context(tc.tile_pool(name="ppA", bufs=2, space="PSUM"))
        pps = ctx.enter_context(tc.tile_pool(name="pps", bufs=2, space="PSUM"))
        ppo = ctx.enter_context(tc.tile_pool(name="ppo", bufs=2, space="PSUM"))

        # ---- constants into SBUF (all bulk loads on the sync HWDGE
        # queue; the gpsimd queue stays clear for the collectives) ----
        def load_w(dram, ncols):
            ts = []
            for d in range(DCH):
                t = consts.tile([128, ncols], BF, tag=f"w{dram.name}{d}")
                nc.sync.dma_start(out=t[:], in_=dram[d * 128:(d + 1) * 128, :])
                ts.append(t)
            return ts

        cos_sb = consts.tile([128, S], BF, tag="cos")
        nc.sync.dma_start(out=cos_sb[:], in_=cosT[:, :])
        sin_sb = consts.tile([128, S], BF, tag="sin")
        nc.sync.dma_start(out=sin_sb[:], in_=sinT[:, :])
        mask_sb = consts.tile([128, KB], BF, tag="mask")
        nc.sync.dma_start(out=mask_sb[:], in_=mask128[:, :])
        ident = consts.tile([128, 128], BF, tag="ident")
        make_identity(nc, ident[:])

        # ---------------- phase builders ----------------
        qt_t, kt_t, v_t, o_t = {}, {}, {}, {}
        xts = {}
        wo_holder = []

        def load_x(b):
            xts[b] = []
            for d in range(DCH):
                t = xpool.tile([128, S], BF, tag="xt")
                nc.sync.dma_start(out=t[:], in_=xT[d * 128:(d + 1) * 128,
                                                  b * S:(b + 1) * S])
                xts[b].append(t)

        def init_batch(b):
            kt_t[b] = pKt.tile([128, S], BF, tag="kt", name=f"kt{b}")
            qt_t[b] = pQt.tile([128, S], BF, tag="qt", name=f"qt{b}")
            v_t[b] = pV.tile([128, NVT, HPC, HD + 1], BF, tag="vt",
                             name=f"vt{b}")
            o_t[b] = pO.tile([128, S], BF, tag="ob", name=f"ob{b}")
            nc.vector.memset(v_t[b][:, :, :, HD:HD + 1], 1.0)

        def proj_mm_st(b, w_sb, raw, st):
            """One 512-col tile of a projection: matmuls + one PSUM
            read into the full-row raw staging tile."""
            ps = ppA.tile([128, QT], F32, tag="pA", name="ps")
            for d in range(DCH):
                nc.tensor.matmul(
                    ps[:], w_sb[d][:], xts[b][d][:, st * QT:(st + 1) * QT],
                    start=(d == 0), stop=(d == DCH - 1))
            c0 = st * QT
            nc.vector.tensor_copy(raw[:, c0:c0 + QT], ps[:])

        def rope_cols(raw, dest, c0, n):
            """RoPE on columns [c0, c0+n) of a full-row raw tile, with
            wide DVE ops to amortize per-op overhead."""
            tcos = ptmp.tile([128, S], BF, tag="tcos")
            nc.vector.tensor_tensor(tcos[:, c0:c0 + n], raw[:, c0:c0 + n],
                                    cos_sb[:, c0:c0 + n], MULT)
            trot = ptmp.tile([128, S], BF, tag="trot")
            for g in range(4):
                o0 = g * 32
                i0 = o0 + 32 if g % 2 == 0 else o0 - 32
                nc.vector.tensor_copy(trot[o0:o0 + 32, c0:c0 + n],
                                      raw[i0:i0 + 32, c0:c0 + n])
            nc.vector.tensor_tensor(trot[:, c0:c0 + n], trot[:, c0:c0 + n],
                                    sin_sb[:, c0:c0 + n], MULT)
            nc.vector.tensor_tensor(dest[:, c0:c0 + n], tcos[:, c0:c0 + n],
                                    trot[:, c0:c0 + n], ADD)

        def proj_v_mm(b, st):
            """V.T projection matmuls for one 512-col tile + PSUM->SBUF
            cast; the PE transpose runs in proj_v_tr (emitted a step
            later so it doesn't stall the PE on the cast)."""
            ps = ppA.tile([128, QT], F32, tag="pA", name="ps")
            for d in range(DCH):
                nc.tensor.matmul(
                    ps[:], wv_sb[d][:], xts[b][d][:, st * QT:(st + 1) * QT],
                    start=(d == 0), stop=(d == DCH - 1))
            vts = ptmp.tile([128, QT], BF, tag="vts")
            nc.vector.tensor_copy(vts[:], ps[:])
            return vts

        def proj_v_tr(b, st, vts):
            """PE-transpose a V.T tile to (kpos, head, hd) vt layout.
            vt column HD holds the ones for the softmax denominator."""
            vt = v_t[b]
            ptr = ppA.tile([128, QT], BF, tag="pA", name="ptr")
            for i in range(QT // 128):
                nc.tensor.transpose(
                    ptr[:, i * 128:(i + 1) * 128],
                    vts[:, i * 128:(i + 1) * 128], ident[:])
            # one strided copy: [kpos, (chunk, head, hd)] -> vt slots
            tsrc = ptr[:].rearrange("p (i h c) -> p i h c", i=4, h=HPC)
            dst = vt[:, st * 4:(st + 1) * 4, :, 0:HD]
            nc.vector.tensor_copy(dst, tsrc)

        def attn_qtile(b, qt_i):
            """Attention for one 512-wide q tile, both local heads."""
            ob = o_t[b]
            q0 = qt_i * QT
            nkb = (q0 + QT) // KB
            po = [ppo.tile([128, QT], F32, tag="po", name=f"po{h}")
                  for h in range(HPC)]
            for kb in range(nkb):
                k0 = kb * KB
                off = k0 - q0 if k0 >= q0 else 0
                ps = pps.tile([128, HPC * QT], F32, tag="ps", name="pp")
                psv = ps[:].rearrange("p (h c) -> p h c", c=QT)
                for h in range(HPC):
                    p0 = h * HD
                    nc.tensor.matmul(
                        psv[:, h, off:QT],
                        kt_t[b][p0:p0 + HD, k0:k0 + KB],
                        qt_t[b][p0:p0 + HD, q0 + off:q0 + QT],
                        start=True, stop=True)
                pt = ppt.tile([128, HPC * QT], BF, tag="pt", name="pt")
                ptv = pt[:].rearrange("p (h c) -> p h c", c=QT)
                nc.scalar.activation(ptv[:, :, off:QT], psv[:, :, off:QT],
                                     EXP, scale=float(HD) ** -0.5)
                if k0 >= q0:   # partial triangle: causal mask
                    for h in range(HPC):
                        nc.vector.tensor_tensor(
                            ptv[:, h, off:off + KB], ptv[:, h, off:off + KB],
                            mask_sb[:], MULT)
                for h in range(HPC):
                    nc.tensor.matmul(
                        po[h][0:HD + 1, off:QT],
                        v_t[b][:, kb, h, 0:HD + 1],
                        ptv[:, h, off:QT],
                        start=(kb == 0), stop=(kb == nkb - 1),
                        skip_group_check=True)
            # normalize: O/l, l = row HD of po.  Launch both heads'
            # recip+broadcast chains first (the DMA flight hides behind
            # the other head's ops); the final multiplies are returned
            # as a closure the schedule emits one weave-step later so
            # the bc-DMA wait doesn't head-of-line block the DVE queue.
            # (reciprocal_approx_fast reading PSUM directly corrupts on
            # HW -- bounce l through SBUF first)
            bcs = []
            for h in range(HPC):
                lsb = pnorm.tile([1, QT], F32, tag="lsb", name="lsb")
                nc.vector.tensor_copy(lsb[:], po[h][HD:HD + 1, :])
                recip32 = pnorm.tile([1, QT], F32, tag="recip32",
                                     name="recip32")
                nc.vector.reciprocal_approx_fast(recip32[:], lsb[:])
                recip = pnorm.tile([1, QT], BF, tag="recip", name="recip")
                nc.vector.tensor_copy(recip[:], recip32[:])
                bc = pnorm.tile([HD, QT], BF, tag="bc", name="bc")
                rr = recip[:]
                nc.sync.dma_start(out=bc[:], in_=bass.AP(
                    tensor=rr.tensor, offset=rr.offset,
                    ap=[[1, 1], [0, HD], rr.ap[-1]]))
                bcs.append(bc)

            def finish():
                for h in range(HPC):
                    p0 = h * HD
                    nc.vector.tensor_tensor(
                        ob[p0:p0 + HD, q0:q0 + QT], po[h][0:HD, :],
                        bcs[h][:], MULT)
            return finish

        def phaseC(b, h):
            """Reshard batch-b rows (half h): heads-split -> row-split."""
            base = h * (S // NHALF)
            for j in range(NC):
                nc.sync.dma_start(
                    out=a2a_in[(b, h)][j, :, :],
                    in_=o_t[b][:, base + j * RHH:base + (j + 1) * RHH])
            nc.gpsimd.collective_compute(
                "AllToAll", mybir.AluOpType.bypass,
                replica_groups=[list(range(NC))],
                ins=[a2a_in[(b, h)][:].opt()], outs=[a2a_out[(b, h)][:].opt()])

        def load_og(b, halves, eng=None):
            """Prefetch the resharded rows for phaseD into SBUF."""
            eng = eng or nc.sync
            og = []
            for d in range(NC):
                t = pog.tile([128, RH], BF, tag="og")
                for i, h in enumerate(halves):
                    eng.dma_start(out=t[:, i * RHH:(i + 1) * RHH],
                                  in_=a2a_out[(b, h)][d, :, :])
                og.append(t)
            return og

        def phaseD(b, halves, og):
            """Out-projection for my rows of batch b (given halves)."""
            wo_sb = wo_holder[0]
            n = RHH * len(halves)
            col0 = b * RH + halves[0] * RHH
            for e in range(DCH):
                ps = ppA.tile([128, QT], F32, tag="pA", name="ps")
                for d in range(NC):
                    nc.tensor.matmul(
                        ps[0:128, 0:n],
                        wo_sb[d][:, e * 128:(e + 1) * 128], og[d][:, 0:n],
                        start=(d == 0), stop=(d == NC - 1))
                osb = posb.tile([128, RH], F32, tag="osb")
                nc.scalar.copy(osb[:, 0:n], ps[0:128, 0:n])
                nc.sync.dma_start(
                    out=out[e * 128:(e + 1) * 128, col0:col0 + n],
                    in_=osb[:, 0:n])

        # ---------------- schedule (emission order biases the
        # dependency-scheduler toward dense PE work; sync-queue DMA
        # order is emission order, so it must never put a long-wait DMA
        # ahead of attention-normalize broadcasts; og prefetches ride
        # the gpsimd queue between collective triggers) ----
        load_x(0)
        wk_sb = load_w(wkT, DPC)
        wv_sb = load_w(wvT, DPC)
        wq_sb = load_w(wqT, DPC)
        load_x(1)
        wo_holder.append(load_w(woT, D))
        init_batch(0)

        def proj_kq(b, w_sb, dest, name, weave=None):
            raw = praw.tile([128, S], BF, tag="raw", name=name)
            proj_mm_st(b, w_sb, raw, 0)
            if weave is not None:
                weave()
            proj_mm_st(b, w_sb, raw, 1)
            rope_cols(raw, dest, 0, 2 * QT)
            proj_mm_st(b, w_sb, raw, 2)
            proj_mm_st(b, w_sb, raw, 3)
            rope_cols(raw, dest, 2 * QT, 2 * QT)

        def proj_v(b):
            vts_q = [proj_v_mm(b, 0), proj_v_mm(b, 1)]
            proj_v_tr(b, 0, vts_q[0])
            vts_q.append(proj_v_mm(b, 2))
            proj_v_tr(b, 1, vts_q[1])
            vts_q.append(proj_v_mm(b, 3))
            proj_v_tr(b, 2, vts_q[2])
            proj_v_tr(b, 3, vts_q[3])

        proj_kq(0, wk_sb, kt_t[0], "rawK0")
        proj_v(0)
        proj_kq(0, wq_sb, qt_t[0], "rawQ0")
        init_batch(1)

        # batch-0 attention woven with batch-1 projections
        fin = attn_qtile(0, 0)
        proj_kq(1, wk_sb, kt_t[1], "rawK1", weave=fin)
        fin = attn_qtile(0, 1)
        fin()
        phaseC(0, 0)
        proj_v(1)
        fin = attn_qtile(0, 2)
        fin()
        fin = attn_qtile(0, 3)
        proj_kq(1, wq_sb, qt_t[1], "rawQ1", weave=fin)
        phaseC(0, 1)

        # batch-1 attention; og prefetches interleave on the gpsimd
        # queue between collective triggers
        fin0 = attn_qtile(1, 0)
        fin1 = attn_qtile(1, 1)
        fin0()
        fin1()
        phaseC(1, 0)
        og0 = load_og(0, [0, 1])
        fin2 = attn_qtile(1, 2)
        og10 = load_og(1, [0])
        fin3 = attn_qtile(1, 3)
        fin2()
        fin3()
        phaseC(1, 1)
        phaseD(0, [0, 1], og0)
        phaseD(1, [0], og10)
        og11 = load_og(1, [1], eng=nc.gpsimd)
        phaseD(1, [1], og11)
    nc.compile()
    return nc


_NC_CACHE = {}


def _get_nc(S):
    if S not in _NC_CACHE:
        _NC_CACHE[S] = build_nc(S)
    return _NC_CACHE[S]


def make_in_maps(x, wq, wk, wv, wo):
    b, S, d = x.shape
    xT = np.ascontiguousarray(x.reshape(b * S, d).T).astype(BF16)
    woT = np.ascontiguousarray(wo.T).astype(BF16)

    # RoPE tables, transposed: partition p -> head-local dim p % 64
    inv = (1.0 / ROPE_BASE ** (np.arange(0, HD, 2, dtype=np.float64) / HD))
    t = np.arange(S, dtype=np.float64)
    fr = np.outer(t, inv)                      # [S, 32]
    emb = np.concatenate([fr, fr], axis=1)     # [S, 64]
    cos_t = np.cos(emb).T                      # [64, S]
    sin_t = np.sin(emb).T
    sgn = np.where(np.arange(HD) < HD // 2, -1.0, 1.0)[:, None]
    cosT = np.concatenate([cos_t, cos_t], axis=0).astype(BF16)       # [128,S]
    sinT = np.concatenate([sin_t * sgn, sin_t * sgn], axis=0).astype(BF16)

    # causal triangle mask for the diagonal 128x128 sub-block:
    # kept iff q >= k  (q = col, k = partition)
    pp = np.arange(128)[:, None]
    qn = np.arange(KB)[None, :]
    mask128 = (qn >= pp).astype(BF16)

    in_maps = []
    for c in range(NC):
        sl = slice(c * DPC, (c + 1) * DPC)
        in_maps.append({
            "xT": xT,
            "wqT": np.ascontiguousarray(wq[sl, :].T).astype(BF16),
            "wkT": np.ascontiguousarray(wk[sl, :].T).astype(BF16),
            "wvT": np.ascontiguousarray(wv[sl, :].T).astype(BF16),
            "woT": woT,
            "cosT": cosT,
            "sinT": sinT,
            "mask128": mask128,
        })
    return in_maps


def assemble(outs, S):
    """outs[c] = per-core (D, B*RH) out.T block -> full (B, S, D)."""
    RH = S // NC
    NHALF = 2
    RHH = RH // NHALF
    outT = np.empty((D, B * S), dtype=np.float32)
    for c in range(NC):
        o = np.asarray(outs[c])
        for bb in range(B):
            for h in range(NHALF):
                g0 = bb * S + h * (S // NHALF) + c * RHH
                l0 = bb * RH + h * RHH
                outT[:, g0:g0 + RHH] = o[:, l0:l0 + RHH]
    return np.ascontiguousarray(outT.T).reshape(B, S, D).astype(np.float32)


def run(x, wq, wk, wv, wo, trace=False):
    b, S, d = x.shape
    nc = _get_nc(S)
    in_maps = make_in_maps(x, wq, wk, wv, wo)
    res = run_bass_kernel_spmd(nc, in_maps, core_ids=list(range(NC)),
                               trace=trace)
    full = assemble([res.results[c]["out"] for c in range(NC)], S)
    return full, res


def kernel(x, wq, wk, wv, wo):
    full, _ = run(np.asarray(x), np.asarray(wq), np.asarray(wk),
                  np.asarray(wv), np.asarray(wo))
    return full


# revision 13
# speedup vs baseline: 1.2792x; 1.2792x over previous
"""Multi-head attention (RoPE + causal SDPA) on 8 Trainium2 NeuronCores.

Sharding: tensor-parallel over heads for QKV+attention (2 heads/core),
then an on-device AllToAll (one per batch half, so comm overlaps other
compute) reshards from head-split to row-split, and each core computes
its row slice of the output projection with the full wo. Host side only
slices/transposes/concatenates.

Everything is computed "transposed" (feature dims on SBUF partitions,
sequence on the free axis), so no transposes are needed on the hot path:
  Qt/Kt  (128=2*64 head dims, S)  = W_slice @ x.T   (lhsT=W_sliceT chunks)
  scores (128 kpos, <=512 q)      lhsT=Kt block, rhs=Qt slice; the two
                                  heads' score matmuls use disjoint PE
                                  row-groups (contraction 64 each) so
                                  they run concurrently, into one
                                  [128, 2*512] PSUM pair tile
  P.T    = exp(scores/8)          one ScalarE ACTIVATE per pair (3D AP)
  O.T    (64+1, <=512 q)          accumulated over k blocks in PSUM;
                                  row 64 = softmax sum (ones col in V)
  out.T  (1024 e, rows)           = woT chunks.T @ O_full.T

Causal structure: k-blocks at/above the q-tile diagonal are trimmed to
the needed column suffix (score MM, exp, PV all shrink); the partial
triangle gets a single [128,128] mask multiply per head.

Emission interleaves batch-1 projections with batch-0 attention (and
out-proj with batch-1 attention) so the PE always has dense independent
work while ScalarE chews exp -- keeps the HAM clock gate at full rate.
"""
import sys, os
if '/opt/trn_rl_repo' not in sys.path:
    sys.path.insert(0, '/opt/trn_rl_repo')
os.environ.setdefault('MYCRO_LOCAL_CACHE', '1')

from contextlib import ExitStack

import numpy as np
import ml_dtypes

import concourse.bass as bass
import concourse.tile as tile
from concourse import bacc, mybir
from concourse.bass_utils import run_bass_kernel_spmd
from concourse.masks import make_identity

BF16 = ml_dtypes.bfloat16
NC = 8           # cores
B = 2            # batch
D = 1024         # model dim
H = 16           # heads
HD = 64          # head dim
HPC = H // NC    # heads per core = 2
DPC = HPC * HD   # head dims per core = 128
ROPE_BASE = 10000.0
QT = 512         # projection tile / attention q tile
KB = 128         # k block (partition axis of score matmuls)

F32 = mybir.dt.float32
BF = mybir.dt.bfloat16
MULT = mybir.AluOpType.mult
ADD = mybir.AluOpType.add
EXP = mybir.ActivationFunctionType.Exp


def build_nc(S):
    """Build+compile the SPMD Bass module for sequence length S."""
    RH = S // NC           # rows per core per batch
    NST = S // QT          # 512-wide seq tiles per batch
    NVT = S // KB          # V tiles (of 128 kpos) per batch
    DCH = D // 128         # contraction chunks (= 8)
    NHALF = 2              # collectives per batch (row halves)
    RHH = RH // NHALF      # rows per core per half

    nc = bacc.Bacc(num_devices=NC)

    xT = nc.declare_dram_parameter("xT", [D, B * S], BF, isOutput=False)
    wqT = nc.declare_dram_parameter("wqT", [D, DPC], BF, isOutput=False)
    wkT = nc.declare_dram_parameter("wkT", [D, DPC], BF, isOutput=False)
    wvT = nc.declare_dram_parameter("wvT", [D, DPC], BF, isOutput=False)
    woT = nc.declare_dram_parameter("woT", [D, D], BF, isOutput=False)
    cosT = nc.declare_dram_parameter("cosT", [128, S], BF, isOutput=False)
    sinT = nc.declare_dram_parameter("sinT", [128, S], BF, isOutput=False)
    mask128 = nc.declare_dram_parameter("mask128", [128, KB], BF,
                                        isOutput=False)
    out = nc.declare_dram_parameter("out", [D, B * RH], F32, isOutput=True)

    a2a_in = {(b, h): nc.dram_tensor(f"a2a_in{b}_{h}", [NC, 128, RHH], BF)
              for b in range(B) for h in range(NHALF)}
    a2a_out = {(b, h): nc.dram_tensor(f"a2a_out{b}_{h}", [NC, 128, RHH], BF)
               for b in range(B) for h in range(NHALF)}

    ctx = ExitStack()
    with ctx:
        tc = ctx.enter_context(tile.TileContext(nc))

        consts = ctx.enter_context(tc.tile_pool(name="consts", bufs=1))
        xpool = ctx.enter_context(tc.tile_pool(name="x", bufs=2 * DCH))
        pQt = ctx.enter_context(tc.tile_pool(name="qt", bufs=2))
        pKt = ctx.enter_context(tc.tile_pool(name="kt", bufs=2))
        pV = ctx.enter_context(tc.tile_pool(name="v", bufs=2))
        pO = ctx.enter_context(tc.tile_pool(name="oall", bufs=2))
        ptmp = ctx.enter_context(tc.tile_pool(name="tmp", bufs=3))
        praw = ctx.enter_context(tc.tile_pool(name="raw", bufs=2))
        ppt = ctx.enter_context(tc.tile_pool(name="pt", bufs=4))
        pnorm = ctx.enter_context(tc.tile_pool(name="norm", bufs=4))
        pog = ctx.enter_context(tc.tile_pool(name="og", bufs=3 * NC))
        posb = ctx.enter_context(tc.tile_pool(name="osb", bufs=2))

        # PSUM: 8 banks total.
        # ppA (2 x 1 bank): projections + PE-transposes + out-proj
        # pps (2 x 2 banks): score pair tiles (128, 2*512) f32
        # ppo (2 x 1 bank): O.T accumulators (128, 512)
        ppA = ctx.enter_context(tc.tile_pool(name="ppA", bufs=2, space="PSUM"))
        pps = ctx.enter_context(tc.tile_pool(name="pps", bufs=2, space="PSUM"))
        ppo = ctx.enter_context(tc.tile_pool(name="ppo", bufs=2, space="PSUM"))

        # ---- constants into SBUF (all bulk loads on the sync HWDGE
        # queue; the gpsimd queue stays clear for the collectives) ----
        def load_w(dram, ncols):
            ts = []
            for d in range(DCH):
                t = consts.tile([128, ncols], BF, tag=f"w{dram.name}{d}")
                nc.sync.dma_start(out=t[:], in_=dram[d * 128:(d + 1) * 128, :])
                ts.append(t)
            return ts

        cos_sb = consts.tile([128, S], BF, tag="cos")
        nc.sync.dma_start(out=cos_sb[:], in_=cosT[:, :])
        sin_sb = consts.tile([128, S], BF, tag="sin")
        nc.sync.dma_start(out=sin_sb[:], in_=sinT[:, :])
        mask_sb = consts.tile([128, KB], BF, tag="mask")
        nc.sync.dma_start(out=mask_sb[:], in_=mask128[:, :])
        ident = consts.tile([128, 128], BF, tag="ident")
        make_identity(nc, ident[:])

        # ---------------- phase builders ----------------
        qt_t, kt_t, v_t, o_t = {}, {}, {}, {}
        xts = {}
        wo_holder = []

        def load_x(b):
            xts[b] = []
            for d in range(DCH):
                t = xpool.tile([128, S], BF, tag="xt")
                nc.sync.dma_start(out=t[:], in_=xT[d * 128:(d + 1) * 128,
                                                  b * S:(b + 1) * S])
                xts[b].append(t)

        def init_batch(b):
            kt_t[b] = pKt.tile([128, S], BF, tag="kt", name=f"kt{b}")
            qt_t[b] = pQt.tile([128, S], BF, tag="qt", name=f"qt{b}")
            v_t[b] = pV.tile([128, NVT, HPC, 2 * HD], BF, tag="vt",
                             name=f"vt{b}")
            o_t[b] = pO.tile([128, S], BF, tag="ob", name=f"ob{b}")
            nc.vector.memset(v_t[b][:, :, :, HD:2 * HD], 1.0)

        def proj_mm_st(b, w_sb, raw, st):
            """One 512-col tile of a projection: matmuls + one PSUM
            read into the full-row raw staging tile."""
            ps = ppA.tile([128, QT], F32, tag="pA", name="ps")
            for d in range(DCH):
                nc.tensor.matmul(
                    ps[:], w_sb[d][:], xts[b][d][:, st * QT:(st + 1) * QT],
                    start=(d == 0), stop=(d == DCH - 1))
            c0 = st * QT
            nc.vector.tensor_copy(raw[:, c0:c0 + QT], ps[:])

        def rope_cols(raw, dest, c0, n):
            """RoPE on columns [c0, c0+n) of a full-row raw tile, with
            wide DVE ops to amortize per-op overhead."""
            tcos = ptmp.tile([128, S], BF, tag="tcos")
            nc.vector.tensor_tensor(tcos[:, c0:c0 + n], raw[:, c0:c0 + n],
                                    cos_sb[:, c0:c0 + n], MULT)
            trot = ptmp.tile([128, S], BF, tag="trot")
            for g in range(4):
                o0 = g * 32
                i0 = o0 + 32 if g % 2 == 0 else o0 - 32
                nc.vector.tensor_copy(trot[o0:o0 + 32, c0:c0 + n],
                                      raw[i0:i0 + 32, c0:c0 + n])
            nc.vector.tensor_tensor(trot[:, c0:c0 + n], trot[:, c0:c0 + n],
                                    sin_sb[:, c0:c0 + n], MULT)
            nc.vector.tensor_tensor(dest[:, c0:c0 + n], tcos[:, c0:c0 + n],
                                    trot[:, c0:c0 + n], ADD)

        def proj_v_mm(b, st):
            """V.T projection matmuls for one 512-col tile + PSUM->SBUF
            cast; the PE transpose runs in proj_v_tr (emitted a step
            later so it doesn't stall the PE on the cast)."""
            ps = ppA.tile([128, QT], F32, tag="pA", name="ps")
            for d in range(DCH):
                nc.tensor.matmul(
                    ps[:], wv_sb[d][:], xts[b][d][:, st * QT:(st + 1) * QT],
                    start=(d == 0), stop=(d == DCH - 1))
            vts = ptmp.tile([128, QT], BF, tag="vts")
            nc.vector.tensor_copy(vts[:], ps[:])
            return vts

        def proj_v_tr(b, st, vts):
            """PE-transpose a V.T tile to (kpos, head, hd) vt layout.
            vt column HD holds the ones for the softmax denominator."""
            vt = v_t[b]
            ptr = ppA.tile([128, QT], BF, tag="pA", name="ptr")
            for i in range(QT // 128):
                nc.tensor.transpose(
                    ptr[:, i * 128:(i + 1) * 128],
                    vts[:, i * 128:(i + 1) * 128], ident[:])
            # one strided copy: [kpos, (chunk, head, hd)] -> vt slots
            tsrc = ptr[:].rearrange("p (i h c) -> p i h c", i=4, h=HPC)
            dst = vt[:, st * 4:(st + 1) * 4, :, 0:HD]
            nc.vector.tensor_copy(dst, tsrc)

        def attn_qtile(b, qt_i):
            """Attention for one 512-wide q tile, both local heads."""
            ob = o_t[b]
            q0 = qt_i * QT
            nkb = (q0 + QT) // KB
            po = [ppo.tile([128, QT], F32, tag="po", name=f"po{h}")
                  for h in range(HPC)]
            for kb in range(nkb):
                k0 = kb * KB
                off = k0 - q0 if k0 >= q0 else 0
                ps = pps.tile([128, HPC * QT], F32, tag="ps", name="pp")
                psv = ps[:].rearrange("p (h c) -> p h c", c=QT)
                for h in range(HPC):
                    p0 = h * HD
                    nc.tensor.matmul(
                        psv[:, h, off:QT],
                        kt_t[b][p0:p0 + HD, k0:k0 + KB],
                        qt_t[b][p0:p0 + HD, q0 + off:q0 + QT],
                        start=True, stop=True)
                pt = ppt.tile([128, HPC * QT], BF, tag="pt", name="pt")
                ptv = pt[:].rearrange("p (h c) -> p h c", c=QT)
                nc.scalar.activation(ptv[:, :, off:QT], psv[:, :, off:QT],
                                     EXP, scale=float(HD) ** -0.5)
                if k0 >= q0:   # partial triangle: causal mask
                    for h in range(HPC):
                        nc.vector.tensor_tensor(
                            ptv[:, h, off:off + KB], ptv[:, h, off:off + KB],
                            mask_sb[:], MULT)
                for h in range(HPC):
                    nc.tensor.matmul(
                        po[h][0:128, off:QT],
                        v_t[b][:, kb, h, :],
                        ptv[:, h, off:QT],
                        start=(kb == 0), stop=(kb == nkb - 1),
                        skip_group_check=True)
            # normalize: O/l.  PV's 64 ones-columns replicated l into po
            # rows 64:128, so the reciprocal is computed directly in
            # broadcast form -- no DMA anywhere in this chain.  The final
            # multiplies are returned as a closure the schedule emits a
            # weave-step later to keep the DVE queue flowing.
            # (reciprocal_approx_fast reading PSUM directly corrupts on
            # HW -- bounce l through SBUF first)
            bcs = []
            for h in range(HPC):
                lsb = pnorm.tile([HD, QT], F32, tag="lsb", name="lsb")
                nc.vector.tensor_copy(lsb[:], po[h][HD:2 * HD, :])
                rec = pnorm.tile([HD, QT], F32, tag="rec", name="rec")
                nc.vector.reciprocal_approx_fast(rec[:], lsb[:])
                bcs.append(rec)

            def finish():
                for h in range(HPC):
                    p0 = h * HD
                    nc.vector.tensor_tensor(
                        ob[p0:p0 + HD, q0:q0 + QT], po[h][0:HD, :],
                        bcs[h][:], MULT)
            return finish

        def phaseC(b, h):
            """Reshard batch-b rows (half h): heads-split -> row-split."""
            base = h * (S // NHALF)
            for j in range(NC):
                nc.sync.dma_start(
                    out=a2a_in[(b, h)][j, :, :],
                    in_=o_t[b][:, base + j * RHH:base + (j + 1) * RHH])
            nc.gpsimd.collective_compute(
                "AllToAll", mybir.AluOpType.bypass,
                replica_groups=[list(range(NC))],
                ins=[a2a_in[(b, h)][:].opt()], outs=[a2a_out[(b, h)][:].opt()])

        def load_og(b, halves, eng=None):
            """Prefetch the resharded rows for phaseD into SBUF."""
            eng = eng or nc.sync
            og = []
            for d in range(NC):
                t = pog.tile([128, RH], BF, tag="og")
                for i, h in enumerate(halves):
                    eng.dma_start(out=t[:, i * RHH:(i + 1) * RHH],
                                  in_=a2a_out[(b, h)][d, :, :])
                og.append(t)
            return og

        def phaseD(b, halves, og):
            """Out-projection for my rows of batch b (given halves)."""
            wo_sb = wo_holder[0]
            n = RHH * len(halves)
            col0 = b * RH + halves[0] * RHH
            for e in range(DCH):
                ps = ppA.tile([128, QT], F32, tag="pA", name="ps")
                for d in range(NC):
                    nc.tensor.matmul(
                        ps[0:128, 0:n],
                        wo_sb[d][:, e * 128:(e + 1) * 128], og[d][:, 0:n],
                        start=(d == 0), stop=(d == NC - 1))
                osb = posb.tile([128, RH], F32, tag="osb")
                nc.scalar.copy(osb[:, 0:n], ps[0:128, 0:n])
                nc.sync.dma_start(
                    out=out[e * 128:(e + 1) * 128, col0:col0 + n],
                    in_=osb[:, 0:n])

        # ---------------- schedule (emission order biases the
        # dependency-scheduler toward dense PE work; sync-queue DMA
        # order is emission order, so it must never put a long-wait DMA
        # ahead of attention-normalize broadcasts; og prefetches ride
        # the gpsimd queue between collective triggers) ----
        load_x(0)
        wk_sb = load_w(wkT, DPC)
        wv_sb = load_w(wvT, DPC)
        wq_sb = load_w(wqT, DPC)
        load_x(1)
        wo_holder.append(load_w(woT, D))
        init_batch(0)

        def proj_kq(b, w_sb, dest, name, weave=None):
            raw = praw.tile([128, S], BF, tag="raw", name=name)
            proj_mm_st(b, w_sb, raw, 0)
            if weave is not None:
                weave()
            proj_mm_st(b, w_sb, raw, 1)
            rope_cols(raw, dest, 0, 2 * QT)
            proj_mm_st(b, w_sb, raw, 2)
            proj_mm_st(b, w_sb, raw, 3)
            rope_cols(raw, dest, 2 * QT, 2 * QT)

        def proj_v(b):
            vts_q = [proj_v_mm(b, 0), proj_v_mm(b, 1)]
            proj_v_tr(b, 0, vts_q[0])
            vts_q.append(proj_v_mm(b, 2))
            proj_v_tr(b, 1, vts_q[1])
            vts_q.append(proj_v_mm(b, 3))
            proj_v_tr(b, 2, vts_q[2])
            proj_v_tr(b, 3, vts_q[3])

        proj_kq(0, wk_sb, kt_t[0], "rawK0")
        proj_v(0)
        proj_kq(0, wq_sb, qt_t[0], "rawQ0")
        init_batch(1)

        # batch-0 attention woven with batch-1 projections
        fin = attn_qtile(0, 0)
        proj_kq(1, wk_sb, kt_t[1], "rawK1", weave=fin)
        fin = attn_qtile(0, 1)
        fin()
        phaseC(0, 0)
        proj_v(1)
        fin = attn_qtile(0, 2)
        fin()
        fin = attn_qtile(0, 3)
        proj_kq(1, wq_sb, qt_t[1], "rawQ1", weave=fin)
        phaseC(0, 1)

        # batch-1 attention; og prefetches interleave on the gpsimd
        # queue between collective triggers
        fin0 = attn_qtile(1, 0)
        fin1 = attn_qtile(1, 1)
        fin0()
        fin1()
        phaseC(1, 0)
        og0 = load_og(0, [0, 1])
        fin2 = attn_qtile(1, 2)
        og10 = load_og(1, [0])
        fin3 = attn_qtile(1, 3)
        fin2()
        fin3()
        phaseC(1, 1)
        phaseD(0, [0, 1], og0)
        phaseD(1, [0], og10)
        og11 = load_og(1, [1], eng=nc.gpsimd)
        phaseD(1, [1], og11)
    nc.compile()
    return nc


_NC_CACHE = {}


def _get_nc(S):
    if S not in _NC_CACHE:
        _NC_CACHE[S] = build_nc(S)
    return _NC_CACHE[S]


def make_in_maps(x, wq, wk, wv, wo):
    b, S, d = x.shape
    xT = np.ascontiguousarray(x.reshape(b * S, d).T).astype(BF16)
    woT = np.ascontiguousarray(wo.T).astype(BF16)

    # RoPE tables, transposed: partition p -> head-local dim p % 64
    inv = (1.0 / ROPE_BASE ** (np.arange(0, HD, 2, dtype=np.float64) / HD))
    t = np.arange(S, dtype=np.float64)
    fr = np.outer(t, inv)                      # [S, 32]
    emb = np.concatenate([fr, fr], axis=1)     # [S, 64]
    cos_t = np.cos(emb).T                      # [64, S]
    sin_t = np.sin(emb).T
    sgn = np.where(np.arange(HD) < HD // 2, -1.0, 1.0)[:, None]
    cosT = np.concatenate([cos_t, cos_t], axis=0).astype(BF16)       # [128,S]
    sinT = np.concatenate([sin_t * sgn, sin_t * sgn], axis=0).astype(BF16)

    # causal triangle mask for the diagonal 128x128 sub-block:
    # kept iff q >= k  (q = col, k = partition)
    pp = np.arange(128)[:, None]
    qn = np.arange(KB)[None, :]
    mask128 = (qn >= pp).astype(BF16)

    in_maps = []
    for c in range(NC):
        sl = slice(c * DPC, (c + 1) * DPC)
        in_maps.append({
            "xT": xT,
            "wqT": np.ascontiguousarray(wq[sl, :].T).astype(BF16),
            "wkT": np.ascontiguousarray(wk[sl, :].T).astype(BF16),
            "wvT": np.ascontiguousarray(wv[sl, :].T).astype(BF16),
            "woT": woT,
            "cosT": cosT,
            "sinT": sinT,
            "mask128": mask128,
        })
    return in_maps


def assemble(outs, S):
    """outs[c] = per-core (D, B*RH) out.T block -> full (B, S, D)."""
    RH = S // NC
    NHALF = 2
    RHH = RH // NHALF
    outT = np.empty((D, B * S), dtype=np.float32)
    for c in range(NC):
        o = np.asarray(outs[c])
        for bb in range(B):
            for h in range(NHALF):
                g0 = bb * S + h * (S // NHALF) + c * RHH
                l0 = bb * RH + h * RHH
                outT[:, g0:g0 + RHH] = o[:, l0:l0 + RHH]
    return np.ascontiguousarray(outT.T).reshape(B, S, D).astype(np.float32)


def run(x, wq, wk, wv, wo, trace=False):
    b, S, d = x.shape
    nc = _get_nc(S)
    in_maps = make_in_maps(x, wq, wk, wv, wo)
    res = run_bass_kernel_spmd(nc, in_maps, core_ids=list(range(NC)),
                               trace=trace)
    full = assemble([res.results[c]["out"] for c in range(NC)], S)
    return full, res


def kernel(x, wq, wk, wv, wo):
    full, _ = run(np.asarray(x), np.asarray(wq), np.asarray(wk),
                  np.asarray(wv), np.asarray(wo))
    return full
